# revision 1
# baseline (speedup 1.0000x reference)
import sys

sys.path.insert(0, "/opt/trn_rl_repo")

import numpy as np

N_CORES = 8
N_EX = 50000
N_KC = 1024
D = 256
SHARD = N_EX // N_CORES
PAD = 6272
BLOCKS = PAD // 128
HALVES = [(0, 3200), (3200, 3072)]
ALPHA = 0.2
WPK = 1808

_CACHE = {}


def _build_nc(sim_safe=False, dbg=()):
    import concourse.bass as bass
    import concourse.mybir as mybir

    f32 = mybir.dt.float32
    bf16 = mybir.dt.bfloat16
    i32 = mybir.dt.int32
    AF = mybir.ActivationFunctionType
    ALU = mybir.AluOpType
    X = mybir.AxisListType.X

    nc = bass.Bass()

    exT_d = nc.declare_dram_parameter("exT", [D, PAD], f32, isOutput=False)
    adjT_d = nc.declare_dram_parameter("adjT", [N_KC, PAD], i32, isOutput=False)
    wpack_d = nc.declare_dram_parameter("wpack", [D, WPK], f32, isOutput=False)
    e_d = nc.declare_dram_parameter("eMat", [D, D], f32, isOutput=False)
    out_d = nc.declare_dram_parameter("out", [PAD, D], f32, isOutput=True)
    exrow_s = nc.dram_tensor("exrow_s", [1, PAD], bf16)

    NG = (BLOCKS + 1) // 2
    adj_tiles = [(h, j) for h in range(2) for j in range(8)]

    from contextlib import ExitStack

    es = ExitStack()
    _ctr = [0]

    def _nm(pfx):
        _ctr[0] += 1
        return f"{pfx}{_ctr[0]}"

    sb = lambda shape, dt: es.enter_context(nc.sbuf_tensor(_nm("t"), shape, dt))
    ps = lambda shape, dt: es.enter_context(nc.psum_tensor(_nm("p"), shape, dt))
    sem = lambda: es.enter_context(nc.semaphore(name=_nm("s")))

    with es:
        wp0 = sb([128, WPK], f32); wp1 = sb([128, WPK], f32)
        exT0 = sb([128, PAD], bf16); exT1 = sb([128, PAD], bf16)
        ebf0 = sb([128, D], bf16); ebf1 = sb([128, D], bf16)
        a2b = sb([128, D], f32)
        w1a1c = sb([128, 2], bf16)
        kcwhE_all = sb([128, 8 * 264], bf16)
        kc_score = sb([128, 8], f32)
        kcs_tmp = sb([128, D], f32)
        ex_row = sb([1, PAD], bf16)
        exb = sb([128, 3200], bf16)
        Lt = sb([128, 3200], f32)
        pm_all = sb([128, 8 * PAD], bf16)
        adjt_all = sb([128, 2 * 3200], bf16)
        recip2 = sb([128, 4], f32)
        ehs2 = sb([128, 3 * D], f32)
        zb2 = sb([128, 2 * 512], f32)
        mb2 = sb([128, 2 * 512], f32)
        e2b = sb([128, 512], f32)
        ps_kcwh = ps([128, D], f32)
        ps_scratch = ps([128, 512], f32)
        ps_att = ps([128, 3 * 512], f32)
        ps_eh = ps([128, 3 * 512], f32)
        (s_d_wp, s_d_misc, s_w1a1t, s_w1a1c, s_kcwh, s_kcj, s_exsc, s_exrow,
         s_bounce, s_exb, s_lrelu, s_exp, s_adj, s_pm, s_blk, s_zdone,
         s_min, s_e2, s_ob, s_store, s_vd, s_adj1) = [sem() for _ in range(22)]
        block = es.enter_context(nc.Block())
        wp = [wp0, wp1]
        exT = [exT0, exT1]
        ebf = [ebf0, ebf1]
        kcwhE = [kcwhE_all[:, 264 * j : 264 * j + 258] for j in range(8)]
        pm = [pm_all[:, PAD * j : PAD * (j + 1)] for j in range(8)]
        adjt = [adjt_all[:, 3200 * k : 3200 * (k + 1)] for k in range(2)]
        ehs = [ehs2[:, D * k : D * (k + 1)] for k in range(3)]
        zb = [zb2[:, 512 * k : 512 * (k + 1)] for k in range(2)]
        mb = [mb2[:, 512 * k : 512 * (k + 1)] for k in range(2)]
        att = [ps_att[:, 512 * k : 512 * k + 258] for k in range(3)]
        eh = [ps_eh[:, 512 * k : 512 * k + D] for k in range(3)]
        ps_w1a1 = ps_scratch[:, 0:2]
        ps_exsc = ps_scratch[0:1, 0:512]
        w1 = [wp[t][:, 0:D] for t in range(2)]
        kchT = [wp[t][:, 2 * D : 2 * D + N_KC] for t in range(2)]
        a1col = [wp[t][:, 1536:1537] for t in range(2)]

        ex_chunks = []
        lo = 0
        while lo < PAD:
            w = min(512, PAD - lo)
            ex_chunks.append((lo, w))
            lo += w
        NCH = len(ex_chunks)

        def half_of(b):
            return 0 if b < 25 else 1

        @block.sync
        def _(sync):
            sync.dma_start(out=wp0[:, :], in_=wpack_d[0:128, :]).then_inc(s_d_wp, 16)
            sync.dma_start(out=wp1[:, :], in_=wpack_d[128:256, :]).then_inc(s_d_wp, 16)
            sync.dma_start(
                out=a2b[:, :],
                in_=wpack_d[0:1, 1537 : 1537 + D].to_broadcast((128, D)),
            ).then_inc(s_d_wp, 16)
            sync.wait_ge(s_exrow, NCH)
            sync.dma_start(out=exrow_s[0:1, :], in_=ex_row[0:1, :]).then_inc(
                s_bounce, 16
            )
            sync.wait_ge(s_bounce, 16)
            sync.dma_start(
                out=exb[:, : HALVES[0][1]],
                in_=exrow_s[0:1, 0 : HALVES[0][1]].to_broadcast((128, HALVES[0][1])),
            ).then_inc(s_exb, 16)
            sync.wait_ge(s_lrelu, 8)
            sync.dma_start(
                out=exb[:, : HALVES[1][1]],
                in_=exrow_s[0:1, HALVES[1][0] : PAD].to_broadcast(
                    (128, HALVES[1][1])
                ),
            ).then_inc(s_exb, 16)
            ns = 0
            for g in range(NG):
                sync.wait_ge(s_ob, g + 1)
                for q in range(2):
                    b = 2 * g + q
                    if b >= BLOCKS:
                        continue
                    sw = 16 if "skip_store" in dbg else 256
                    sync.dma_start(
                        out=out_d[128 * b : 128 * b + 1, :sw].rearrange("a b -> a b") if False else out_d[128 * b : 128 * (b + 1), :sw],
                        in_=mb2[
                            :,
                            512 * (g % 2) + 256 * q : 512 * (g % 2) + 256 * q + sw,
                        ],
                    ).then_inc(s_store, 16)
                    ns += 1
            sync.wait_ge(s_store, 16 * ns)

        @block.gpsimd
        def _(gp):
            gp.dma_start(out=ebf0[:, :], in_=e_d[0:128, :]).then_inc(s_d_misc, 16)
            gp.dma_start(out=ebf1[:, :], in_=e_d[128:256, :]).then_inc(s_d_misc, 16)
            xw = 64 if "skip_exTdma" in dbg else PAD
            gp.dma_start(out=exT0[:, :xw], in_=exT_d[0:128, :xw]).then_inc(s_d_misc, 16)
            gp.dma_start(out=exT1[:, :xw], in_=exT_d[128:256, :xw]).then_inc(s_d_misc, 16)

            def issue_adj(idx):
                h, j = adj_tiles[idx]
                hlo, hw = HALVES[h]
                dw = 64 if "skip_adjdma" in dbg else hw
                gp.dma_start(
                    out=adjt[idx % 2][:, :dw],
                    in_=adjT_d[128 * j : 128 * (j + 1), hlo : hlo + dw],
                ).then_inc(s_adj if idx % 2 == 0 else s_adj1, 16)

            issue_adj(0)
            issue_adj(1)
            for idx in range(16):
                h, j = adj_tiles[idx]
                hlo, hw = HALVES[h]
                gp.wait_ge(s_adj if idx % 2 == 0 else s_adj1, 16 * (idx // 2 + 1))
                gp.wait_ge(s_exp, idx + 1)
                if "skip_mask" in dbg:
                    gp.tensor_tensor(
                        out=pm[j][:, hlo : hlo + 64],
                        in0=pm[j][:, hlo : hlo + 64],
                        in1=adjt[idx % 2][:, :64],
                        op=ALU.mult,
                    ).then_inc(s_pm, 1)
                else:
                    gp.tensor_tensor(
                        out=pm[j][:, hlo : hlo + hw],
                        in0=pm[j][:, hlo : hlo + hw],
                        in1=adjt[idx % 2][:, :hw],
                        op=ALU.mult,
                    ).then_inc(s_pm, 1)
                if idx + 2 < 16:
                    gp.wait_ge(s_pm, idx + 1)
                    issue_adj(idx + 2)

        @block.tensor
        def _(pe):
            pe.wait_ge(s_d_wp, 48)
            for t in range(2):
                for kt in range(2):
                    mm = nc.tensor.matmul(
                        ps_w1a1[:, t : t + 1] if False else ps_scratch[:, t : t + 1],
                        wp[kt][:, D + 128 * t : D + 128 * (t + 1)],
                        a1col[kt],
                        start=(kt == 0),
                        stop=(kt == 1),
                    )
                    if t == 1 and kt == 1:
                        mm.then_inc(s_w1a1t, 1)
            for j in range(8):
                if j >= 1:
                    pe.wait_ge(s_kcj, j)
                for t in range(2):
                    mm = nc.tensor.matmul(
                        ps_kcwh[:, :],
                        kchT[t][:, 128 * j : 128 * (j + 1)],
                        w1[t],
                        start=(t == 0),
                        stop=(t == 1),
                    )
                    if t == 1:
                        mm.then_inc(s_kcwh, 1)
            pe.wait_ge(s_w1a1c, 1)
            pe.wait_ge(s_d_misc, 64)
            for s, (lo, w) in enumerate(ex_chunks):
                if s >= 1:
                    pe.wait_ge(s_exrow, s)
                for t in range(2):
                    mm = nc.tensor.matmul(
                        ps_scratch[0:1, :w],
                        w1a1c[:, t : t + 1],
                        exT[t][:, lo : lo + w],
                        start=(t == 0),
                        stop=(t == 1),
                    )
                    if t == 1:
                        mm.then_inc(s_exsc, 1)
            pe.wait_ge(s_kcj, 8)
            for b in range(BLOCKS):
                k = b % 3
                pe.wait_ge(s_pm, 8 if half_of(b) == 0 else 16)
                if b >= 3:
                    pe.wait_ge(s_zdone, b - 2)
                aw2 = 16 if "skip_attmm" in dbg else 258
                for j in range(8):
                    nc.tensor.matmul(
                        att[k][:, 0:aw2],
                        pm[j][:, 128 * b : 128 * (b + 1)],
                        kcwhE[j][:, 0:aw2],
                        start=(j == 0),
                        stop=(j == 7),
                    )
                ew2 = 16 if "skip_ehmm" in dbg else D
                for t in range(2):
                    mm = nc.tensor.matmul(
                        eh[k][:, 0:ew2],
                        exT[t][:, 128 * b : 128 * (b + 1)],
                        ebf[t][:, 0:ew2],
                        start=(t == 0),
                        stop=(t == 1),
                    )
                    if t == 1:
                        mm.then_inc(s_blk, 1)

        @block.vector
        def _(dv):
            vd_n = [0]
            dv.wait_ge(s_d_wp, 48)
            for j in range(8):
                dv.wait_ge(s_kcwh, j + 1)
                nc.vector.tensor_copy(out=kcwhE[j][:, 0:D], in_=ps_kcwh[:, :])
                nc.vector.memset(kcwhE[j][:, D : D + 1], 1.0)
                nc.vector.memset(kcwhE[j][:, D + 1 : D + 2], 0.0)
                nc.vector.tensor_tensor(
                    out=kcs_tmp[:, :], in0=ps_kcwh[:, :], in1=a2b[:, :], op=ALU.mult
                ).then_inc(s_vd, 1)
                vd_n[0] += 1
                dv.wait_ge(s_vd, vd_n[0])
                nc.vector.reduce_sum(
                    kc_score[:, j : j + 1], kcs_tmp[:, :], axis=X
                ).then_inc(s_kcj, 1)
            dv.wait_ge(s_w1a1t, 1)
            nc.vector.tensor_copy(out=w1a1c[:, :], in_=ps_scratch[:, 0:2]).then_inc(
                s_w1a1c, 1
            )
            for s, (lo, w) in enumerate(ex_chunks):
                dv.wait_ge(s_exsc, s + 1)
                nc.vector.tensor_copy(
                    out=ex_row[0:1, lo : lo + w], in_=ps_scratch[0:1, :w]
                ).then_inc(s_exrow, 1)

            def out_stt(g2):
                w2 = 512 if 2 * g2 + 1 < BLOCKS else 256
                if "skip_epi" in dbg:
                    w2 = 16
                dv.wait_ge(s_min, g2 + 1)
                dv.wait_ge(s_e2, g2 + 1)
                nc.vector.scalar_tensor_tensor(
                    out=mb[g2 % 2][:, :w2],
                    in0=e2b[:, :w2],
                    scalar=-1.0,
                    in1=zb[g2 % 2][:, :w2],
                    op0=ALU.add,
                    op1=ALU.max,
                ).then_inc(s_ob, 1)

            pending = []
            for b in range(BLOCKS):
                k = b % 3
                g, q = divmod(b, 2)
                dv.wait_ge(s_blk, b + 1)
                ew = 16 if "skip_epi" in dbg else D
                nc.vector.reciprocal(recip2[:, k : k + 1], att[k][:, D : D + 1])
                nc.vector.tensor_copy(out=ehs[k][:, :ew], in_=eh[k][:, :ew]).then_inc(s_vd, 1)
                vd_n[0] += 1
                dv.wait_ge(s_vd, vd_n[0])
                nc.vector.scalar_tensor_tensor(
                    out=zb[g % 2][:, 256 * q : 256 * q + ew],
                    in0=att[k][:, 0:ew],
                    scalar=recip2[:, k : k + 1],
                    in1=ehs[k][:, :ew],
                    op0=ALU.mult,
                    op1=ALU.mult,
                ).then_inc(s_zdone, 1)
                if (q == 1) or (b == BLOCKS - 1):
                    w = 256 * (q + 1)
                    if g >= 2:
                        done_blocks = min(2 * (g - 1), BLOCKS)
                        dv.wait_ge(s_store, 16 * done_blocks)
                    dv.wait_ge(s_zdone, min(2 * g + 2, BLOCKS))
                    if "skip_epi" in dbg:
                        w = 16
                    nc.vector.tensor_scalar_min(
                        mb[g % 2][:, :w], zb[g % 2][:, :w], 0.0
                    ).then_inc(s_min, 1)
                    pending.append(g)
                    if len(pending) >= 2:
                        out_stt(pending.pop(0))
            for g2 in pending:
                out_stt(g2)

        @block.scalar
        def _(act):
            lr_n = [0]
            ex_n = [0]

            def score_item(h, j):
                hlo, hw = HALVES[h]
                act.wait_ge(s_exb, 16 * (h + 1))
                act.wait_ge(s_kcj, j + 1)
                if ex_n[0]:
                    act.wait_ge(s_exp, ex_n[0])
                aw = 64 if "skip_act" in dbg else hw
                nc.scalar.activation(
                    Lt[:, :aw],
                    exb[:, :aw],
                    AF.Relu if sim_safe else AF.Prelu,
                    bias=kc_score[:, j : j + 1],
                    scale=1.0,
                    alpha=ALPHA,
                ).then_inc(s_lrelu, 1)
                lr_n[0] += 1
                act.wait_ge(s_lrelu, lr_n[0])
                nc.scalar.activation(
                    pm[j][:, hlo : hlo + aw], Lt[:, :aw], AF.Exp
                ).then_inc(s_exp, 1)
                ex_n[0] += 1

            def elu_item(g):
                w = 512 if 2 * g + 1 < BLOCKS else 256
                act.wait_ge(s_min, g + 1)
                if g >= 1:
                    act.wait_ge(s_ob, g)
                nc.scalar.activation(e2b[:, :w], mb[g % 2][:, :w], AF.Exp).then_inc(
                    s_e2, 1
                )

            for j in range(8):
                score_item(0, j)
            gq = 0
            for j in range(8):
                score_item(1, j)
                if gq < 4:
                    elu_item(gq)
                    gq += 1
            for g in range(gq, NG):
                elu_item(g)

    return nc


def _prep_shards(exercise_h, kc_h, adj_exercise_kc, W1, E, a):
    exercise_h = np.asarray(exercise_h, dtype=np.float32)
    kc_h = np.asarray(kc_h, dtype=np.float32)
    adj = np.asarray(adj_exercise_kc, dtype=np.int32)
    W1 = np.asarray(W1, dtype=np.float32)
    E = np.asarray(E, dtype=np.float32)
    a = np.asarray(a, dtype=np.float32)

    wpack = np.zeros((D, WPK), dtype=np.float32)
    wpack[:, 0:D] = W1
    wpack[:, D : 2 * D] = W1.T
    wpack[:, 2 * D : 2 * D + N_KC] = kc_h.T
    wpack[:, 1536] = a[:D, 0]
    wpack[0, 1537 : 1537 + D] = a[D:, 0]
    wpack = np.ascontiguousarray(wpack)

    in_maps = []
    for i in range(N_CORES):
        lo = i * SHARD
        exT = np.zeros((D, PAD), dtype=np.float32)
        exT[:, :SHARD] = exercise_h[lo : lo + SHARD].T
        adjT = np.zeros((N_KC, PAD), dtype=np.int32)
        adjT[:, :SHARD] = adj[lo : lo + SHARD].T
        adjT[0, SHARD:] = 1
        in_maps.append(
            {
                "exT": np.ascontiguousarray(exT),
                "adjT": np.ascontiguousarray(adjT),
                "wpack": wpack,
                "eMat": E,
            }
        )
    return in_maps


def kernel(exercise_h, kc_h, adj_exercise_kc, W1, E, a, _trace=False, _tmpdir=None):
    from concourse.bass_utils import run_bass_kernel_spmd

    if "nc" not in _CACHE:
        _CACHE["nc"] = _build_nc()
    nc = _CACHE["nc"]

    in_maps = _prep_shards(exercise_h, kc_h, adj_exercise_kc, W1, E, a)
    res = run_bass_kernel_spmd(
        nc, in_maps, list(range(N_CORES)), trace=_trace, tmpdir=_tmpdir
    )
    _CACHE["last_result"] = res
    out = np.concatenate(
        [np.asarray(res.results[i]["out"])[:SHARD] for i in range(N_CORES)], axis=0
    )
    return out.astype(np.float32)



# revision 17
# speedup vs baseline: 1.1422x; 1.1422x over previous
import sys

sys.path.insert(0, "/opt/trn_rl_repo")

import numpy as np

N_CORES = 8
N_EX = 50000
N_KC = 1024
D = 256
SHARD = N_EX // N_CORES
PAD = 6272
BLOCKS = PAD // 128
HALVES = [(0, 3200), (3200, 3072)]
ALPHA = 0.2
WPK = 1808

_CACHE = {}


def _build_nc(sim_safe=False, dbg=()):
    import concourse.bass as bass
    import concourse.mybir as mybir

    f32 = mybir.dt.float32
    bf16 = mybir.dt.bfloat16
    i32 = mybir.dt.int32
    AF = mybir.ActivationFunctionType
    ALU = mybir.AluOpType
    X = mybir.AxisListType.X

    nc = bass.Bass()

    exT_d = nc.declare_dram_parameter("exT", [D, PAD], f32, isOutput=False)
    adjT_d = nc.declare_dram_parameter("adjT", [N_KC, PAD], i32, isOutput=False)
    wpack_d = nc.declare_dram_parameter("wpack", [D, WPK], f32, isOutput=False)
    e_d = nc.declare_dram_parameter("eMat", [D, D], f32, isOutput=False)
    out_d = nc.declare_dram_parameter("out", [PAD, D], f32, isOutput=True)
    exrow_s = nc.dram_tensor("exrow_s", [1, PAD], bf16)

    NG = (BLOCKS + 1) // 2
    adj_tiles = [(h, j) for h in range(2) for j in range(8)]

    from contextlib import ExitStack

    es = ExitStack()
    _ctr = [0]

    def _nm(pfx):
        _ctr[0] += 1
        return f"{pfx}{_ctr[0]}"

    sb = lambda shape, dt: es.enter_context(nc.sbuf_tensor(_nm("t"), shape, dt))
    ps = lambda shape, dt: es.enter_context(nc.psum_tensor(_nm("p"), shape, dt))
    sem = lambda: es.enter_context(nc.semaphore(name=_nm("s")))

    with es:
        wp0 = sb([128, WPK], f32); wp1 = sb([128, WPK], f32)
        exT0 = sb([128, PAD], bf16); exT1 = sb([128, PAD], bf16)
        ebf0 = sb([128, D], bf16); ebf1 = sb([128, D], bf16)
        a2b = sb([128, D], f32)
        w1a1c = sb([128, 2], bf16)
        kcwhE_all = sb([128, 8 * 264], bf16)
        kc_score = sb([128, 8], f32)
        kcs_tmp = sb([128, D], f32)
        ex_row = sb([1, PAD], bf16)
        exb = sb([128, 3200], bf16)
        Lt = sb([128, 3200], f32)
        pm_all = sb([128, 8 * PAD], bf16)
        adjt_all = sb([128, 2 * 3200], bf16)
        recip2 = sb([128, 4], f32)
        ehs2 = sb([128, 3 * D], f32)
        zb2 = sb([128, 2 * 512], f32)
        mb2 = sb([128, 2 * 512], f32)
        e2b = sb([128, 512], f32)
        ps_kcwh = ps([128, D], f32)
        ps_scratch = ps([128, 512], f32)
        ps_att = ps([128, 3 * 512], f32)
        ps_eh = ps([128, 3 * 512], f32)
        (s_d_wp, s_d_misc, s_w1a1t, s_w1a1c, s_kcwh, s_kcj, s_exsc, s_exrow,
         s_bounce, s_exb, s_lrelu, s_exp, s_adj, s_pm, s_blk, s_zdone,
         s_min, s_e2, s_ob, s_store, s_vd, s_adj1) = [sem() for _ in range(22)]
        block = es.enter_context(nc.Block())
        wp = [wp0, wp1]
        exT = [exT0, exT1]
        ebf = [ebf0, ebf1]
        kcwhE = [kcwhE_all[:, 264 * j : 264 * j + 258] for j in range(8)]
        pm = [pm_all[:, PAD * j : PAD * (j + 1)] for j in range(8)]
        adjt = [adjt_all[:, 3200 * k : 3200 * (k + 1)] for k in range(2)]
        ehs = [ehs2[:, D * k : D * (k + 1)] for k in range(3)]
        zb = [zb2[:, 512 * k : 512 * (k + 1)] for k in range(2)]
        mb = [mb2[:, 512 * k : 512 * (k + 1)] for k in range(2)]
        att = [ps_att[:, 512 * k : 512 * k + 258] for k in range(3)]
        eh = [ps_eh[:, 512 * k : 512 * k + D] for k in range(3)]
        ps_w1a1 = ps_scratch[:, 0:2]
        ps_exsc = ps_scratch[0:1, 0:512]
        w1 = [wp[t][:, 0:D] for t in range(2)]
        kchT = [wp[t][:, 2 * D : 2 * D + N_KC] for t in range(2)]
        a1col = [wp[t][:, 1536:1537] for t in range(2)]

        ex_chunks = []
        lo = 0
        while lo < PAD:
            w = min(512, PAD - lo)
            ex_chunks.append((lo, w))
            lo += w
        NCH = len(ex_chunks)

        def half_of(b):
            return 0 if b < 25 else 1

        @block.sync
        def _(sync):
            sync.dma_start(out=wp0[:, :], in_=wpack_d[0:128, :]).then_inc(s_d_wp, 16)
            sync.dma_start(out=wp1[:, :], in_=wpack_d[128:256, :]).then_inc(s_d_wp, 16)
            sync.dma_start(
                out=a2b[:, :],
                in_=wpack_d[0:1, 1537 : 1537 + D].to_broadcast((128, D)),
            ).then_inc(s_d_wp, 16)
            sync.wait_ge(s_exrow, NCH)
            sync.dma_start(out=exrow_s[0:1, :], in_=ex_row[0:1, :]).then_inc(
                s_bounce, 16
            )
            sync.wait_ge(s_bounce, 16)
            sync.dma_start(
                out=exb[:, : HALVES[0][1]],
                in_=exrow_s[0:1, 0 : HALVES[0][1]].to_broadcast((128, HALVES[0][1])),
            ).then_inc(s_exb, 16)
            sync.wait_ge(s_lrelu, 8)
            sync.dma_start(
                out=exb[:, : HALVES[1][1]],
                in_=exrow_s[0:1, HALVES[1][0] : PAD].to_broadcast(
                    (128, HALVES[1][1])
                ),
            ).then_inc(s_exb, 16)
            ns = 0
            for g in range(NG):
                sync.wait_ge(s_ob, g + 1)
                for q in range(2):
                    b = 2 * g + q
                    if b >= BLOCKS:
                        continue
                    sw = 16 if "skip_store" in dbg else 256
                    sync.dma_start(
                        out=out_d[128 * b : 128 * b + 1, :sw].rearrange("a b -> a b") if False else out_d[128 * b : 128 * (b + 1), :sw],
                        in_=mb2[
                            :,
                            512 * (g % 2) + 256 * q : 512 * (g % 2) + 256 * q + sw,
                        ],
                    ).then_inc(s_store, 16)
                    ns += 1
            sync.wait_ge(s_store, 16 * ns)

        @block.gpsimd
        def _(gp):
            gp.dma_start(out=ebf0[:, :], in_=e_d[0:128, :]).then_inc(s_d_misc, 16)
            gp.dma_start(out=ebf1[:, :], in_=e_d[128:256, :]).then_inc(s_d_misc, 16)
            xw = 64 if "skip_exTdma" in dbg else PAD
            gp.dma_start(out=exT0[:, :xw], in_=exT_d[0:128, :xw]).then_inc(s_d_misc, 16)
            gp.dma_start(out=exT1[:, :xw], in_=exT_d[128:256, :xw]).then_inc(s_d_misc, 16)

            def issue_adj(idx):
                h, j = adj_tiles[idx]
                hlo, hw = HALVES[h]
                dw = 64 if "skip_adjdma" in dbg else hw
                gp.dma_start(
                    out=adjt[idx % 2][:, :dw],
                    in_=adjT_d[128 * j : 128 * (j + 1), hlo : hlo + dw],
                ).then_inc(s_adj if idx % 2 == 0 else s_adj1, 16)

            issue_adj(0)
            issue_adj(1)
            for idx in range(16):
                h, j = adj_tiles[idx]
                hlo, hw = HALVES[h]
                gp.wait_ge(s_adj if idx % 2 == 0 else s_adj1, 16 * (idx // 2 + 1))
                gp.wait_ge(s_exp, idx + 1)
                if "skip_mask" in dbg:
                    gp.tensor_tensor(
                        out=pm[j][:, hlo : hlo + 64],
                        in0=pm[j][:, hlo : hlo + 64],
                        in1=adjt[idx % 2][:, :64],
                        op=ALU.mult,
                    ).then_inc(s_pm, 1)
                else:
                    gp.tensor_tensor(
                        out=pm[j][:, hlo : hlo + hw],
                        in0=pm[j][:, hlo : hlo + hw],
                        in1=adjt[idx % 2][:, :hw],
                        op=ALU.mult,
                    ).then_inc(s_pm, 1)
                if idx + 2 < 16:
                    gp.wait_ge(s_pm, idx + 1)
                    issue_adj(idx + 2)

        @block.tensor
        def _(pe):
            pe.wait_ge(s_d_wp, 48)
            for t in range(2):
                for kt in range(2):
                    mm = nc.tensor.matmul(
                        ps_w1a1[:, t : t + 1] if False else ps_scratch[:, t : t + 1],
                        wp[kt][:, D + 128 * t : D + 128 * (t + 1)],
                        a1col[kt],
                        start=(kt == 0),
                        stop=(kt == 1),
                    )
                    if t == 1 and kt == 1:
                        mm.then_inc(s_w1a1t, 1)
            for j in range(8):
                if j >= 1:
                    pe.wait_ge(s_kcj, j)
                for t in range(2):
                    mm = nc.tensor.matmul(
                        ps_kcwh[:, :],
                        kchT[t][:, 128 * j : 128 * (j + 1)],
                        w1[t],
                        start=(t == 0),
                        stop=(t == 1),
                    )
                    if t == 1:
                        mm.then_inc(s_kcwh, 1)
            pe.wait_ge(s_w1a1c, 1)
            pe.wait_ge(s_d_misc, 64)
            for s, (lo, w) in enumerate(ex_chunks):
                if s >= 1:
                    pe.wait_ge(s_exrow, s)
                for t in range(2):
                    mm = nc.tensor.matmul(
                        ps_scratch[0:1, :w],
                        w1a1c[:, t : t + 1],
                        exT[t][:, lo : lo + w],
                        start=(t == 0),
                        stop=(t == 1),
                    )
                    if t == 1:
                        mm.then_inc(s_exsc, 1)
            pe.wait_ge(s_kcj, 8)
            for b in range(BLOCKS):
                k = b % 3
                pe.wait_ge(s_pm, 8 if half_of(b) == 0 else 16)
                if b >= 3:
                    pe.wait_ge(s_zdone, b - 2)
                aw2 = 16 if "skip_attmm" in dbg else 258
                for j in range(8):
                    nc.tensor.matmul(
                        att[k][:, 0:aw2],
                        pm[j][:, 128 * b : 128 * (b + 1)],
                        kcwhE[j][:, 0:aw2],
                        start=(j == 0),
                        stop=(j == 7),
                    )
                ew2 = 16 if "skip_ehmm" in dbg else D
                for t in range(2):
                    mm = nc.tensor.matmul(
                        eh[k][:, 0:ew2],
                        exT[t][:, 128 * b : 128 * (b + 1)],
                        ebf[t][:, 0:ew2],
                        start=(t == 0),
                        stop=(t == 1),
                    )
                    if t == 1:
                        mm.then_inc(s_blk, 1)

        @block.vector
        def _(dv):
            vd_n = [0]
            dv.wait_ge(s_d_wp, 48)
            for j in range(8):
                dv.wait_ge(s_kcwh, j + 1)
                nc.vector.tensor_copy(out=kcwhE[j][:, 0:D], in_=ps_kcwh[:, :])
                nc.vector.memset(kcwhE[j][:, D : D + 1], 1.0)
                nc.vector.memset(kcwhE[j][:, D + 1 : D + 2], 0.0)
                nc.vector.tensor_tensor(
                    out=kcs_tmp[:, :], in0=ps_kcwh[:, :], in1=a2b[:, :], op=ALU.mult
                ).then_inc(s_vd, 1)
                vd_n[0] += 1
                dv.wait_ge(s_vd, vd_n[0])
                nc.vector.reduce_sum(
                    kc_score[:, j : j + 1], kcs_tmp[:, :], axis=X
                ).then_inc(s_kcj, 1)
            dv.wait_ge(s_w1a1t, 1)
            nc.vector.tensor_copy(out=w1a1c[:, :], in_=ps_scratch[:, 0:2]).then_inc(
                s_w1a1c, 1
            )
            for s, (lo, w) in enumerate(ex_chunks):
                dv.wait_ge(s_exsc, s + 1)
                nc.vector.tensor_copy(
                    out=ex_row[0:1, lo : lo + w], in_=ps_scratch[0:1, :w]
                ).then_inc(s_exrow, 1)

            def out_stt(g2):
                w2 = 512 if 2 * g2 + 1 < BLOCKS else 256
                if "skip_epi" in dbg:
                    w2 = 16
                dv.wait_ge(s_min, g2 + 1)
                dv.wait_ge(s_e2, g2 + 1)
                nc.vector.scalar_tensor_tensor(
                    out=mb[g2 % 2][:, :w2],
                    in0=e2b[:, :w2],
                    scalar=-1.0,
                    in1=zb[g2 % 2][:, :w2],
                    op0=ALU.add,
                    op1=ALU.max,
                ).then_inc(s_ob, 1)

            pending = []
            for b in range(BLOCKS):
                k = b % 3
                g, q = divmod(b, 2)
                dv.wait_ge(s_blk, b + 1)
                ew = 16 if "skip_epi" in dbg else D
                nc.vector.reciprocal(recip2[:, k : k + 1], att[k][:, D : D + 1])
                nc.vector.tensor_copy(out=ehs[k][:, :ew], in_=eh[k][:, :ew]).then_inc(s_vd, 1)
                vd_n[0] += 1
                dv.wait_ge(s_vd, vd_n[0])
                nc.vector.scalar_tensor_tensor(
                    out=zb[g % 2][:, 256 * q : 256 * q + ew],
                    in0=att[k][:, 0:ew],
                    scalar=recip2[:, k : k + 1],
                    in1=ehs[k][:, :ew],
                    op0=ALU.mult,
                    op1=ALU.mult,
                ).then_inc(s_zdone, 1)
                if (q == 1) or (b == BLOCKS - 1):
                    w = 256 * (q + 1)
                    if g >= 2:
                        done_blocks = min(2 * (g - 1), BLOCKS)
                        dv.wait_ge(s_store, 16 * done_blocks)
                    dv.wait_ge(s_zdone, min(2 * g + 2, BLOCKS))
                    if "skip_epi" in dbg:
                        w = 16
                    nc.vector.tensor_scalar_min(
                        mb[g % 2][:, :w], zb[g % 2][:, :w], 0.0
                    ).then_inc(s_min, 1)
                    pending.append(g)
                    if len(pending) >= 2:
                        out_stt(pending.pop(0))
            for g2 in pending:
                out_stt(g2)

        @block.scalar
        def _(act):
            lr_n = [0]
            ex_n = [0]

            def score_item(h, j):
                hlo, hw = HALVES[h]
                act.wait_ge(s_exb, 16 * (h + 1))
                act.wait_ge(s_kcj, j + 1)
                if ex_n[0]:
                    act.wait_ge(s_exp, ex_n[0])
                aw = 64 if "skip_act" in dbg else hw
                nc.scalar.activation(
                    Lt[:, :aw],
                    exb[:, :aw],
                    AF.Relu if sim_safe else AF.Prelu,
                    bias=kc_score[:, j : j + 1],
                    scale=1.0,
                    alpha=ALPHA,
                ).then_inc(s_lrelu, 1)
                lr_n[0] += 1
                act.wait_ge(s_lrelu, lr_n[0])
                nc.scalar.activation(
                    pm[j][:, hlo : hlo + aw], Lt[:, :aw], AF.Exp
                ).then_inc(s_exp, 1)
                ex_n[0] += 1

            def elu_item(g):
                w = 512 if 2 * g + 1 < BLOCKS else 256
                act.wait_ge(s_min, g + 1)
                if g >= 1:
                    act.wait_ge(s_ob, g)
                nc.scalar.activation(e2b[:, :w], mb[g % 2][:, :w], AF.Exp).then_inc(
                    s_e2, 1
                )

            for j in range(8):
                score_item(0, j)
            gq = 0
            for j in range(8):
                score_item(1, j)
                if gq < 4:
                    elu_item(gq)
                    gq += 1
            for g in range(gq, NG):
                elu_item(g)

    return nc


def _prep_shards(exercise_h, kc_h, adj_exercise_kc, W1, E, a):
    exercise_h = np.asarray(exercise_h, dtype=np.float32)
    kc_h = np.asarray(kc_h, dtype=np.float32)
    adj = np.asarray(adj_exercise_kc, dtype=np.int32)
    W1 = np.asarray(W1, dtype=np.float32)
    E = np.asarray(E, dtype=np.float32)
    a = np.asarray(a, dtype=np.float32)

    wpack = np.zeros((D, WPK), dtype=np.float32)
    wpack[:, 0:D] = W1
    wpack[:, D : 2 * D] = W1.T
    wpack[:, 2 * D : 2 * D + N_KC] = kc_h.T
    wpack[:, 1536] = a[:D, 0]
    wpack[0, 1537 : 1537 + D] = a[D:, 0]
    wpack = np.ascontiguousarray(wpack)

    in_maps = []
    for i in range(N_CORES):
        lo = i * SHARD
        exT = np.zeros((D, PAD), dtype=np.float32)
        exT[:, :SHARD] = exercise_h[lo : lo + SHARD].T
        adjT = np.zeros((N_KC, PAD), dtype=np.int32)
        adjT[:, :SHARD] = adj[lo : lo + SHARD].T
        adjT[0, SHARD:] = 1
        in_maps.append(
            {
                "exT": np.ascontiguousarray(exT),
                "adjT": np.ascontiguousarray(adjT),
                "wpack": wpack,
                "eMat": E,
            }
        )
    return in_maps


def kernel(exercise_h, kc_h, adj_exercise_kc, W1, E, a, _trace=False, _tmpdir=None):
    from concourse.bass_utils import run_bass_kernel_spmd

    if "nc" not in _CACHE:
        _CACHE["nc"] = _build_nc()
    nc = _CACHE["nc"]

    in_maps = _prep_shards(exercise_h, kc_h, adj_exercise_kc, W1, E, a)
    res = run_bass_kernel_spmd(
        nc, in_maps, list(range(N_CORES)), trace=_trace, tmpdir=_tmpdir
    )
    _CACHE["last_result"] = res
    out = np.concatenate(
        [np.asarray(res.results[i]["out"])[:SHARD] for i in range(N_CORES)], axis=0
    )
    return out.astype(np.float32)

